# revision 4
# baseline (speedup 1.0000x reference)
"""Trainium2 Bass kernel for nn_AttentionResBlock (windowed causal attention +
sigmoid*tanh gating + two 1x1 convs), SPMD over 8 NeuronCores.

Sharding: data-parallel over (batch, sequence-half): core i handles batch i//2,
rows [h*2048, (h+1)*2048) with h = i%2, plus a 512-row halo (previous window;
zeros + mask flag for h==0). No cross-core communication.

Host prepares both layouts of x (natural [t,c] tiles AND transposed [c,t]
tiles) pre-arranged so every DMA is a plain contiguous burst — no on-device
transposes, no DMA-transpose mode, no 512B-descriptor stalls. The [ones,zeros]
pad columns (softmax-denominator trick) are baked into the natural tiles.

Per-core pipeline (window = 512 queries, kv = 1024 keys):
  scoresT[j,q] = kvT^T @ qT      (PE bf16, softmax scale folded into exp)
  expT = exp(scale*scoresT)      (ACT, PSUM->SBUF, bf16 out)
  causal mask: affine_select fill=0 on diagonal j-chunks; halo flag multiply
  o_unnorm[q, c+2] = sum_j expT[j,q]^T @ [kv | 1 | 0]  (PE; col 256 = denom)
  o = o_unnorm[:, :256] * recip(denom)   (DVE per-partition scalar)
  oT via PE transpose into PSUM; gating reads PSUM directly:
  u = sigmoid(a) * tanh(a)               (2x ACT + 1 DVE mul)
  res/skip[t,d] = u^T @ [Wr|Ws]^T        (PE, fused along N; GpSimd PSUM->SBUF
      bf16 copy), per-window batched DMA out in a pre-tiled layout the host
      unshuffles. Biases are added on the host (they are zero in this model).

The emission is software-pipelined with a one-window lag so the in-order
engine queues run window w+1's attention while ACT/DVE finish window w's
gating/projections. A few identity matmuls at the head of the Tensor queue
warm the HAM clock-gate during the initial DMA shadow.
"""

import numpy as np

B, T, C = 4, 4096, 256
W = 512                # attention window
TCH = T // 2           # rows per core
TH = TCH + W           # with halo
NWIN = TCH // W        # windows per core (4)
NBLK = TH // W         # 512-row blocks per core (5)
NCORES = 8

_CACHE = {}


def _build_program():
    import concourse.bacc as bacc
    import concourse.bass as bass
    import concourse.mybir as mybir
    import concourse.tile as tile
    from concourse.masks import make_identity

    f32 = mybir.dt.float32
    qdt = mybir.dt.bfloat16
    ts = bass.ts

    nc = bacc.Bacc("TRN2", target_bir_lowering=False, debug=False)

    xt_d = nc.dram_tensor("xt", [NBLK, 128, 2, W], qdt, kind="ExternalInput").ap()
    xn_d = nc.dram_tensor("xn", [NBLK, 128, 4, C + 2], qdt, kind="ExternalInput").ap()
    wc_d = nc.dram_tensor("wc", [128, 2, 2 * C], qdt, kind="ExternalInput").ap()
    hflag = nc.dram_tensor("hflag", [128, 1], f32, kind="ExternalInput").ap()
    rs_d = nc.dram_tensor("rs", [NWIN, 128, 4, 2 * C], qdt, kind="ExternalOutput").ap()

    Exp = mybir.ActivationFunctionType.Exp
    Tanh = mybir.ActivationFunctionType.Tanh
    Sig = mybir.ActivationFunctionType.Sigmoid

    with tile.TileContext(nc) as tc:
        with (
            tc.tile_pool(name="singles", bufs=1) as singles,
            tc.tile_pool(name="xn", bufs=5) as xn_pool,
            tc.tile_pool(name="xt", bufs=5) as xt_pool,
            tc.tile_pool(name="ex", bufs=16) as ex_pool,
            tc.tile_pool(name="on", bufs=6) as on_pool,
            tc.tile_pool(name="g", bufs=3) as g_pool,
            tc.tile_pool(name="outs", bufs=3) as out_pool,
            tc.tile_pool(name="small", bufs=8) as small,
            tc.tile_pool(name="psc", bufs=3, space="PSUM") as sc_pool,
            tc.tile_pool(name="pav", bufs=3, space="PSUM") as avj_pool,
            tc.tile_pool(name="pt", bufs=2, space="PSUM") as pt_pool,
        ):
            hf_sb = singles.tile([128, 1], f32)
            wc_sb = singles.tile([128, 2, 2 * C], qdt)
            xnb = [
                xn_pool.tile([128, 4, C + 2], qdt, tag="xn", name=f"xn{i}")
                for i in range(NBLK)
            ]
            xtb = [
                xt_pool.tile([128, 2, W], qdt, tag="xt", name=f"xt{i}")
                for i in range(NBLK)
            ]

            # weights/flag ride the scalar queue; x streams on sync, ordered
            # by first use. QK(0) needs blk1 (q) + blk0,1 (kv): cc0 halves
            # first so the cc=0 accumulation half starts earliest.
            nc.scalar.dma_start(out=wc_sb, in_=wc_d)
            nc.scalar.dma_start(out=hf_sb, in_=hflag)
            nc.sync.dma_start(out=xtb[1][:, 0, :], in_=xt_d[1, :, 0, :])
            nc.sync.dma_start(out=xtb[0][:, 0, :], in_=xt_d[0, :, 0, :])
            nc.sync.dma_start(out=xtb[1][:, 1, :], in_=xt_d[1, :, 1, :])
            nc.sync.dma_start(out=xtb[0][:, 1, :], in_=xt_d[0, :, 1, :])
            nc.sync.dma_start(out=xnb[0], in_=xn_d[0])
            nc.sync.dma_start(out=xnb[1], in_=xn_d[1])
            for blk in (2, 3, 4):
                nc.sync.dma_start(out=xtb[blk], in_=xt_d[blk])
                nc.sync.dma_start(out=xnb[blk], in_=xn_d[blk])

            identf = singles.tile([128, 128], f32)
            make_identity(nc, identf)
            ident = singles.tile([128, 128], qdt)
            nc.vector.tensor_copy(ident, identf)
            # hoist the ACT table loads into the DMA shadow
            actwarm = singles.tile([128, 3], f32)
            nc.scalar.activation(out=actwarm[:, 0:1], in_=identf[:, 0:1], func=Exp)
            nc.scalar.activation(out=actwarm[:, 1:2], in_=identf[:, 0:1], func=Tanh)
            nc.scalar.activation(out=actwarm[:, 2:3], in_=identf[:, 0:1], func=Sig)
            # HAM clock-gate warm-up: real-rate matmuls on the identity while
            # the first x tiles are still in flight
            pwarm = pt_pool.tile([128, 128], f32, tag="pt")
            for k in range(4):
                nc.tensor.matmul(pwarm, ident, ident, start=(k == 0), stop=(k == 3))

            def attn_stage(w):
                """scores -> exp -> mask -> AV -> normalize -> oT (PSUM)."""
                qt = xtb[w + 1]

                # ---- scoresT[j, q] = (kv @ q^T) per j-chunk; exp; mask ----
                # chunks 0..5 full q; 6,7 only q in [256,512), one shared bank
                expts = [None] * 8  # (ap, q_lo) per j-chunk
                for jc in range(6):
                    q_lo = 128 if jc == 5 else 0  # q < 128 fully masked for 5
                    kvt = xtb[w + jc // 4]
                    psc = sc_pool.tile([128, W - q_lo], f32, tag="sc")
                    for cc in range(2):
                        nc.tensor.matmul(
                            psc,
                            kvt[:, cc, ts(jc % 4, 128)],
                            qt[:, cc, q_lo:W],
                            start=(cc == 0),
                            stop=(cc == 1),
                        )
                    ex = ex_pool.tile([128, W - q_lo], qdt, tag="ex2")
                    nc.scalar.activation(out=ex, in_=psc, func=Exp, scale=0.0625)
                    expts[jc] = (ex, q_lo)
                # 6,7 share one PSUM bank as a single accumulation group
                kvt = xtb[w + 1]
                psc = sc_pool.tile([128, 2, 256], f32, tag="sc")
                for i, jc in enumerate((6, 7)):
                    for cc in range(2):
                        nc.tensor.matmul(
                            psc[:, i, :],
                            kvt[:, cc, ts(jc % 4, 128)],
                            qt[:, cc, 256:512],
                            start=(i == 0 and cc == 0),
                            stop=(i == 1 and cc == 1),
                        )
                ex67 = ex_pool.tile([128, 2, 256], qdt, tag="ex1")
                nc.scalar.activation(out=ex67, in_=psc, func=Exp, scale=0.0625)
                expts[6] = (ex67[:, 0, :], 256)
                expts[7] = (ex67[:, 1, :], 256)

                # causal mask: valid iff q - p + 512 - jc*128 >= 0
                for jc in (4, 5):
                    ap, q_lo = expts[jc]
                    nc.gpsimd.affine_select(
                        out=ap,
                        in_=ap,
                        compare_op=mybir.AluOpType.is_ge,
                        fill=0.0,
                        base=q_lo + W - jc * 128,
                        channel_multiplier=-1,
                        pattern=[[1, W - q_lo]],
                    )
                nc.gpsimd.affine_select(
                    out=ex67,
                    in_=ex67,
                    compare_op=mybir.AluOpType.is_ge,
                    fill=0.0,
                    base=0,
                    channel_multiplier=-1,
                    pattern=[[-128, 2], [1, 256]],
                )
                if w == 0:
                    # halo validity flag (1.0 = real halo, 0.0 = first window)
                    for jc in range(4):
                        nc.vector.tensor_scalar_mul(
                            expts[jc][0], expts[jc][0], hf_sb
                        )

                # ---- AV + denom; normalize; transpose to oT (PSUM) ----
                pt4 = pt_pool.tile([128, 2, W], qdt, tag="pt")
                ons = []
                for qb in range(4):
                    jcs = [
                        jc
                        for jc in range(8)
                        if not (qb * 128 + 127) < (jc * 128 - W)
                    ]
                    pav = avj_pool.tile([128, C + 2], f32, tag="av")
                    for k, jc in enumerate(jcs):
                        ap, q_lo = expts[jc]
                        xn = xnb[w + jc // 4]
                        nc.tensor.matmul(
                            pav,
                            ap[:, qb * 128 - q_lo : qb * 128 - q_lo + 128],
                            xn[:, jc % 4, :],
                            start=(k == 0),
                            stop=(k == len(jcs) - 1),
                        )
                    rc = small.tile([128, 1], f32, tag="rc")
                    nc.vector.reciprocal(rc, pav[:, C : C + 1])
                    on = on_pool.tile([128, C], qdt, tag="on")
                    nc.vector.tensor_scalar_mul(on, pav[:, 0:C], rc)
                    ons.append(on)
                for qb in range(4):
                    for cc in range(2):
                        nc.tensor.transpose(
                            pt4[:, cc, ts(qb, 128)], ons[qb][:, ts(cc, 128)], ident
                        )
                return pt4

            def out_stage(w, pt4, last=False):
                """gating -> projections -> store, for window w."""
                sg = g_pool.tile([128, 2, W], qdt, tag="sg")
                ta = g_pool.tile([128, 2, W], qdt, tag="ta")
                nc.scalar.activation(out=sg, in_=pt4, func=Sig)
                nc.scalar.activation(out=ta, in_=pt4, func=Tanh)
                nc.vector.tensor_mul(sg, ta, sg)
                us = [sg[:, 0, :], sg[:, 1, :]]
                rs_win = out_pool.tile([128, 4, 2 * C], qdt, tag="rs")
                for qb in range(4):
                    psp = avj_pool.tile([128, 2 * C], f32, tag="av")
                    for cc in range(2):
                        nc.tensor.matmul(
                            psp,
                            us[cc][:, ts(qb, 128)],
                            wc_sb[:, cc, :],
                            start=(cc == 0),
                            stop=(cc == 1),
                        )
                    # PSUM->SBUF move (GpSimd/DMA can't touch PSUM): split
                    # between DVE and ACT so neither becomes the bottleneck
                    if qb % 2 == 0:
                        nc.vector.tensor_copy(rs_win[:, qb, :], psp)
                    else:
                        nc.scalar.activation(
                            out=rs_win[:, qb, :],
                            in_=psp,
                            func=mybir.ActivationFunctionType.Copy,
                        )
                    if last:
                        # final window: store per q-block so the DMA overlaps
                        # the remaining projections instead of the drain tail
                        q = nc.sync if qb % 2 == 0 else nc.scalar
                        q.dma_start(out=rs_d[w, :, qb, :], in_=rs_win[:, qb, :])
                if not last:
                    nc.sync.dma_start(out=rs_d[w], in_=rs_win)

            # software pipeline with a one-window lag
            pts = {}
            pts[0] = attn_stage(0)
            for w in range(1, NWIN):
                pts[w] = attn_stage(w)
                out_stage(w - 1, pts.pop(w - 1))
            out_stage(NWIN - 1, pts.pop(NWIN - 1), last=True)

    nc.compile()
    return nc


def _get_program():
    if "nc" not in _CACHE:
        _CACHE["nc"] = _build_program()
    return _CACHE["nc"]


def _make_in_maps(x, Wr, br, Ws, bs):
    import ml_dtypes

    bf16 = ml_dtypes.bfloat16
    x = np.asarray(x, dtype=np.float32)
    Wr = np.asarray(Wr, dtype=np.float32)
    Ws = np.asarray(Ws, dtype=np.float32)

    # res/skip projections fused along the output dim; [c, cc-chunked] layout
    wcomb = np.concatenate([Wr.T, Ws.T], axis=1)          # [C, 2C]
    wc_h = np.ascontiguousarray(
        wcomb.reshape(2, 128, 2 * C).transpose(1, 0, 2)
    ).astype(bf16)

    in_maps = []
    for i in range(NCORES):
        b, h = divmod(i, 2)
        xhf = np.empty((TH, C), np.float32)
        if h == 0:
            xhf[:W] = 0.0
            flag = np.zeros((128, 1), np.float32)
        else:
            xhf[:W] = x[b, TCH - W : TCH]
            flag = np.ones((128, 1), np.float32)
        xhf[W:] = x[b, h * TCH : (h + 1) * TCH]
        xb = xhf.astype(bf16)
        # transposed tiles: xt[blk, p, cc, t] = x[blk*512 + t, cc*128 + p]
        xt_h = np.ascontiguousarray(
            xb.reshape(NBLK, W, 2, 128).transpose(0, 3, 2, 1)
        )
        # natural tiles + baked [ones, zeros] pad columns (denominator trick)
        xn_core = xb.reshape(NBLK, 4, 128, C).transpose(0, 2, 1, 3)
        xn_h = np.empty((NBLK, 128, 4, C + 2), bf16)
        xn_h[:, :, :, 0:C] = xn_core
        xn_h[:, :, :, C] = bf16(1.0)
        xn_h[:, :, :, C + 1] = bf16(0.0)
        in_maps.append(
            {
                "xt": xt_h,
                "xn": np.ascontiguousarray(xn_h),
                "wc": wc_h,
                "hflag": flag,
            }
        )
    return in_maps


def _gather(results, br, bs):
    residual = np.empty((B, T, C), np.float32)
    skip = np.empty((B, T, C), np.float32)
    for i in range(NCORES):
        b, h = divmod(i, 2)
        rs = np.asarray(results[i]["rs"], dtype=np.float32)  # [4, 128, 4, 2C]
        rs = rs.transpose(0, 2, 1, 3).reshape(TCH, 2 * C)
        residual[b, h * TCH : (h + 1) * TCH] = rs[:, 0:C]
        skip[b, h * TCH : (h + 1) * TCH] = rs[:, C : 2 * C]
    br = np.asarray(br, dtype=np.float32)
    bs = np.asarray(bs, dtype=np.float32)
    if br.any():
        residual += br
    if bs.any():
        skip += bs
    return residual, skip


def kernel(x, Wr, br, Ws, bs):
    from concourse.bass_utils import run_bass_kernel_spmd

    nc = _get_program()
    in_maps = _make_in_maps(x, Wr, br, Ws, bs)
    res = run_bass_kernel_spmd(nc, in_maps, list(range(NCORES)))
    return _gather(res.results, br, bs)


# revision 8
# speedup vs baseline: 1.0074x; 1.0074x over previous
"""Trainium2 Bass kernel for nn_AttentionResBlock (windowed causal attention +
sigmoid*tanh gating + two 1x1 convs), SPMD over 8 NeuronCores.

Sharding: data-parallel over (batch, sequence-half): core i handles batch i//2,
rows [h*2048, (h+1)*2048) with h = i%2, plus a 512-row halo (previous window;
zeros + mask flag for h==0). No cross-core communication.

Host prepares both layouts of x (natural [t,c] tiles AND transposed [c,t]
tiles) pre-arranged so every DMA is a plain contiguous burst — no on-device
transposes, no DMA-transpose mode, no 512B-descriptor stalls. The [ones,zeros]
pad columns (softmax-denominator trick) are baked into the natural tiles.

Per-core pipeline (window = 512 queries, kv = 1024 keys):
  scoresT[j,q] = kvT^T @ qT      (PE bf16, softmax scale folded into exp)
  expT = exp(scale*scoresT)      (ACT, PSUM->SBUF, bf16 out)
  causal mask: affine_select fill=0 on diagonal j-chunks; halo flag multiply
  o_unnorm[q, c+2] = sum_j expT[j,q]^T @ [kv | 1 | 0]  (PE; col 256 = denom)
  o = o_unnorm[:, :256] * recip(denom)   (DVE per-partition scalar)
  oT via PE transpose into PSUM; gating reads PSUM directly:
  u = sigmoid(a) * tanh(a)               (2x ACT + 1 DVE mul)
  res/skip[t,d] = u^T @ [Wr|Ws]^T        (PE, fused along N; GpSimd PSUM->SBUF
      bf16 copy), per-window batched DMA out in a pre-tiled layout the host
      unshuffles. Biases are added on the host (they are zero in this model).

The emission is software-pipelined with a one-window lag so the in-order
engine queues run window w+1's attention while ACT/DVE finish window w's
gating/projections. A few identity matmuls at the head of the Tensor queue
warm the HAM clock-gate during the initial DMA shadow.
"""

import numpy as np

B, T, C = 4, 4096, 256
W = 512                # attention window
TCH = T // 2           # rows per core
TH = TCH + W           # with halo
NWIN = TCH // W        # windows per core (4)
NBLK = TH // W         # 512-row blocks per core (5)
NCORES = 8

_CACHE = {}


def _build_program():
    import concourse.bacc as bacc
    import concourse.bass as bass
    import concourse.mybir as mybir
    import concourse.tile as tile
    from concourse.masks import make_identity

    f32 = mybir.dt.float32
    qdt = mybir.dt.bfloat16
    ts = bass.ts

    nc = bacc.Bacc("TRN2", target_bir_lowering=False, debug=False)

    xt_d = nc.dram_tensor("xt", [NBLK, 128, 2, W], qdt, kind="ExternalInput").ap()
    xn_d = nc.dram_tensor("xn", [NBLK, 128, 4, C + 2], qdt, kind="ExternalInput").ap()
    wc_d = nc.dram_tensor("wc", [128, 2, 2 * C], qdt, kind="ExternalInput").ap()
    hflag = nc.dram_tensor("hflag", [128, 1], f32, kind="ExternalInput").ap()
    rs_d = nc.dram_tensor("rs", [NWIN, 128, 4, 2 * C], qdt, kind="ExternalOutput").ap()

    Exp = mybir.ActivationFunctionType.Exp
    Tanh = mybir.ActivationFunctionType.Tanh
    Sig = mybir.ActivationFunctionType.Sigmoid

    with tile.TileContext(nc) as tc:
        with (
            tc.tile_pool(name="singles", bufs=1) as singles,
            tc.tile_pool(name="xn", bufs=5) as xn_pool,
            tc.tile_pool(name="xt", bufs=5) as xt_pool,
            tc.tile_pool(name="ex", bufs=16) as ex_pool,
            tc.tile_pool(name="on", bufs=6) as on_pool,
            tc.tile_pool(name="g", bufs=3) as g_pool,
            tc.tile_pool(name="outs", bufs=3) as out_pool,
            tc.tile_pool(name="small", bufs=8) as small,
            tc.tile_pool(name="psc", bufs=3, space="PSUM") as sc_pool,
            tc.tile_pool(name="pav", bufs=3, space="PSUM") as avj_pool,
            tc.tile_pool(name="pt", bufs=2, space="PSUM") as pt_pool,
        ):
            hf_sb = singles.tile([128, 1], f32)
            wc_sb = singles.tile([128, 2, 2 * C], qdt)
            xnb = [
                xn_pool.tile([128, 4, C + 2], qdt, tag="xn", name=f"xn{i}")
                for i in range(NBLK)
            ]
            xtb = [
                xt_pool.tile([128, 2, W], qdt, tag="xt", name=f"xt{i}")
                for i in range(NBLK)
            ]

            # weights/flag ride the scalar queue; x streams on sync, ordered
            # by first use. QK(0) needs blk1 (q) + blk0,1 (kv): cc0 halves
            # first so the cc=0 accumulation half starts earliest.
            nc.scalar.dma_start(out=wc_sb, in_=wc_d)
            nc.scalar.dma_start(out=hf_sb, in_=hflag)
            nc.sync.dma_start(out=xtb[1][:, 0, :], in_=xt_d[1, :, 0, :])
            nc.sync.dma_start(out=xtb[0][:, 0, :], in_=xt_d[0, :, 0, :])
            nc.sync.dma_start(out=xtb[1][:, 1, :], in_=xt_d[1, :, 1, :])
            nc.sync.dma_start(out=xtb[0][:, 1, :], in_=xt_d[0, :, 1, :])
            nc.sync.dma_start(out=xnb[0], in_=xn_d[0])
            nc.sync.dma_start(out=xnb[1], in_=xn_d[1])
            for blk in (2, 3, 4):
                nc.sync.dma_start(out=xtb[blk], in_=xt_d[blk])
                nc.sync.dma_start(out=xnb[blk], in_=xn_d[blk])

            identf = singles.tile([128, 128], f32)
            make_identity(nc, identf)
            ident = singles.tile([128, 128], qdt)
            nc.vector.tensor_copy(ident, identf)
            # hoist the ACT table load into the DMA shadow (Exp and Tanh
            # share a table set; Sigmoid does NOT — using it thrashes
            # ACT_TABLE_LOADs, so sigmoid comes from the tanh identity)
            actwarm = singles.tile([128, 2], f32)
            nc.scalar.activation(out=actwarm[:, 0:1], in_=identf[:, 0:1], func=Exp)
            nc.scalar.activation(out=actwarm[:, 1:2], in_=identf[:, 0:1], func=Tanh)
            # HAM clock-gate warm-up: real-rate matmuls on the identity while
            # the first x tiles are still in flight
            pwarm = pt_pool.tile([128, 128], f32, tag="pt")
            for k in range(4):
                nc.tensor.matmul(pwarm, ident, ident, start=(k == 0), stop=(k == 3))

            def attn_stage(w):
                """scores -> exp -> mask -> AV -> normalize -> oT (PSUM)."""
                qt = xtb[w + 1]

                # ---- scoresT[j, q] = (kv @ q^T) per j-chunk; exp; mask ----
                # chunks 0..5 full q; 6,7 only q in [256,512), one shared bank
                expts = [None] * 8  # (ap, q_lo) per j-chunk
                for jc in range(6):
                    q_lo = 128 if jc == 5 else 0  # q < 128 fully masked for 5
                    kvt = xtb[w + jc // 4]
                    psc = sc_pool.tile([128, W - q_lo], f32, tag="sc")
                    for cc in range(2):
                        nc.tensor.matmul(
                            psc,
                            kvt[:, cc, ts(jc % 4, 128)],
                            qt[:, cc, q_lo:W],
                            start=(cc == 0),
                            stop=(cc == 1),
                        )
                    ex = ex_pool.tile([128, W - q_lo], qdt, tag="ex2")
                    nc.scalar.activation(out=ex, in_=psc, func=Exp, scale=0.0625)
                    expts[jc] = (ex, q_lo)
                # 6,7 share one PSUM bank as a single accumulation group
                kvt = xtb[w + 1]
                psc = sc_pool.tile([128, 2, 256], f32, tag="sc")
                for i, jc in enumerate((6, 7)):
                    for cc in range(2):
                        nc.tensor.matmul(
                            psc[:, i, :],
                            kvt[:, cc, ts(jc % 4, 128)],
                            qt[:, cc, 256:512],
                            start=(i == 0 and cc == 0),
                            stop=(i == 1 and cc == 1),
                        )
                ex67 = ex_pool.tile([128, 2, 256], qdt, tag="ex1")
                nc.scalar.activation(out=ex67, in_=psc, func=Exp, scale=0.0625)
                expts[6] = (ex67[:, 0, :], 256)
                expts[7] = (ex67[:, 1, :], 256)

                # causal mask: valid iff q - p + 512 - jc*128 >= 0
                for jc in (4, 5):
                    ap, q_lo = expts[jc]
                    nc.gpsimd.affine_select(
                        out=ap,
                        in_=ap,
                        compare_op=mybir.AluOpType.is_ge,
                        fill=0.0,
                        base=q_lo + W - jc * 128,
                        channel_multiplier=-1,
                        pattern=[[1, W - q_lo]],
                    )
                nc.gpsimd.affine_select(
                    out=ex67,
                    in_=ex67,
                    compare_op=mybir.AluOpType.is_ge,
                    fill=0.0,
                    base=0,
                    channel_multiplier=-1,
                    pattern=[[-128, 2], [1, 256]],
                )
                if w == 0:
                    # halo validity flag (1.0 = real halo, 0.0 = first window)
                    for jc in range(4):
                        nc.vector.tensor_scalar_mul(
                            expts[jc][0], expts[jc][0], hf_sb
                        )

                # ---- AV + denom; normalize; transpose to oT (PSUM) ----
                pt4 = pt_pool.tile([128, 2, W], qdt, tag="pt")
                ons = []
                for qb in range(4):
                    jcs = [
                        jc
                        for jc in range(8)
                        if not (qb * 128 + 127) < (jc * 128 - W)
                    ]
                    pav = avj_pool.tile([128, C + 2], f32, tag="av")
                    for k, jc in enumerate(jcs):
                        ap, q_lo = expts[jc]
                        xn = xnb[w + jc // 4]
                        nc.tensor.matmul(
                            pav,
                            ap[:, qb * 128 - q_lo : qb * 128 - q_lo + 128],
                            xn[:, jc % 4, :],
                            start=(k == 0),
                            stop=(k == len(jcs) - 1),
                        )
                    rc = small.tile([128, 1], f32, tag="rc")
                    nc.vector.reciprocal(rc, pav[:, C : C + 1])
                    on = on_pool.tile([128, C], qdt, tag="on")
                    nc.vector.tensor_scalar_mul(on, pav[:, 0:C], rc)
                    ons.append(on)
                for qb in range(4):
                    for cc in range(2):
                        nc.tensor.transpose(
                            pt4[:, cc, ts(qb, 128)], ons[qb][:, ts(cc, 128)], ident
                        )
                return pt4

            def out_stage(w, pt4, last=False):
                """gating -> projections -> store, for window w."""
                # u = tanh(a) + tanh(a)*tanh(a/2); the 0.5 from
                # sigmoid(a) = (1+tanh(a/2))/2 lives in the host-side weights
                th2 = g_pool.tile([128, 2, W], qdt, tag="th2")
                ta = g_pool.tile([128, 2, W], qdt, tag="ta")
                nc.scalar.activation(out=th2, in_=pt4, func=Tanh, scale=0.5)
                nc.scalar.activation(out=ta, in_=pt4, func=Tanh)
                nc.vector.tensor_mul(th2, ta, th2)
                nc.gpsimd.tensor_add(th2, ta, th2)
                us = [th2[:, 0, :], th2[:, 1, :]]
                rs_win = out_pool.tile([128, 4, 2 * C], qdt, tag="rs")
                for qb in range(4):
                    psp = avj_pool.tile([128, 2 * C], f32, tag="av")
                    for cc in range(2):
                        nc.tensor.matmul(
                            psp,
                            us[cc][:, ts(qb, 128)],
                            wc_sb[:, cc, :],
                            start=(cc == 0),
                            stop=(cc == 1),
                        )
                    # PSUM->SBUF move (GpSimd/DMA can't touch PSUM): mostly
                    # DVE, one per window on ACT to balance engine load
                    if qb == 1:
                        nc.scalar.activation(
                            out=rs_win[:, qb, :],
                            in_=psp,
                            func=mybir.ActivationFunctionType.Copy,
                        )
                    else:
                        nc.vector.tensor_copy(rs_win[:, qb, :], psp)
                    if last:
                        # final window: store per q-block so the DMA overlaps
                        # the remaining projections instead of the drain tail
                        q = nc.sync if qb % 2 == 0 else nc.scalar
                        q.dma_start(out=rs_d[w, :, qb, :], in_=rs_win[:, qb, :])
                if not last:
                    nc.sync.dma_start(out=rs_d[w], in_=rs_win)

            # software pipeline with a one-window lag
            pts = {}
            pts[0] = attn_stage(0)
            for w in range(1, NWIN):
                pts[w] = attn_stage(w)
                out_stage(w - 1, pts.pop(w - 1))
            out_stage(NWIN - 1, pts.pop(NWIN - 1), last=True)

    nc.compile()
    return nc


def _get_program():
    if "nc" not in _CACHE:
        _CACHE["nc"] = _build_program()
    return _CACHE["nc"]


def _make_in_maps(x, Wr, br, Ws, bs):
    import ml_dtypes

    bf16 = ml_dtypes.bfloat16
    x = np.asarray(x, dtype=np.float32)
    Wr = np.asarray(Wr, dtype=np.float32)
    Ws = np.asarray(Ws, dtype=np.float32)

    # 0.5x from the sigmoid(a) = (1 + tanh(a/2))/2 identity folded into
    # weights; res/skip projections fused along the output dim
    wcomb = 0.5 * np.concatenate([Wr.T, Ws.T], axis=1)    # [C, 2C]
    wc_h = np.ascontiguousarray(
        wcomb.reshape(2, 128, 2 * C).transpose(1, 0, 2)
    ).astype(bf16)

    in_maps = []
    for i in range(NCORES):
        b, h = divmod(i, 2)
        xhf = np.empty((TH, C), np.float32)
        if h == 0:
            xhf[:W] = 0.0
            flag = np.zeros((128, 1), np.float32)
        else:
            xhf[:W] = x[b, TCH - W : TCH]
            flag = np.ones((128, 1), np.float32)
        xhf[W:] = x[b, h * TCH : (h + 1) * TCH]
        xb = xhf.astype(bf16)
        # transposed tiles: xt[blk, p, cc, t] = x[blk*512 + t, cc*128 + p]
        xt_h = np.ascontiguousarray(
            xb.reshape(NBLK, W, 2, 128).transpose(0, 3, 2, 1)
        )
        # natural tiles + baked [ones, zeros] pad columns (denominator trick)
        xn_core = xb.reshape(NBLK, 4, 128, C).transpose(0, 2, 1, 3)
        xn_h = np.empty((NBLK, 128, 4, C + 2), bf16)
        xn_h[:, :, :, 0:C] = xn_core
        xn_h[:, :, :, C] = bf16(1.0)
        xn_h[:, :, :, C + 1] = bf16(0.0)
        in_maps.append(
            {
                "xt": xt_h,
                "xn": np.ascontiguousarray(xn_h),
                "wc": wc_h,
                "hflag": flag,
            }
        )
    return in_maps


def _gather(results, br, bs):
    residual = np.empty((B, T, C), np.float32)
    skip = np.empty((B, T, C), np.float32)
    for i in range(NCORES):
        b, h = divmod(i, 2)
        rs = np.asarray(results[i]["rs"], dtype=np.float32)  # [4, 128, 4, 2C]
        rs = rs.transpose(0, 2, 1, 3).reshape(TCH, 2 * C)
        residual[b, h * TCH : (h + 1) * TCH] = rs[:, 0:C]
        skip[b, h * TCH : (h + 1) * TCH] = rs[:, C : 2 * C]
    br = np.asarray(br, dtype=np.float32)
    bs = np.asarray(bs, dtype=np.float32)
    if br.any():
        residual += br
    if bs.any():
        skip += bs
    return residual, skip


def kernel(x, Wr, br, Ws, bs):
    from concourse.bass_utils import run_bass_kernel_spmd

    nc = _get_program()
    in_maps = _make_in_maps(x, Wr, br, Ws, bs)
    res = run_bass_kernel_spmd(nc, in_maps, list(range(NCORES)))
    return _gather(res.results, br, bs)


# revision 10
# speedup vs baseline: 1.1066x; 1.0985x over previous
"""Trainium2 Bass kernel for nn_AttentionResBlock (windowed causal attention +
sigmoid*tanh gating + two 1x1 convs), SPMD over 8 NeuronCores.

Sharding: data-parallel over (batch, sequence-half): core i handles batch i//2,
rows [h*2048, (h+1)*2048) with h = i%2, plus a 512-row halo (previous window;
zeros + mask flag for h==0). No cross-core communication.

Host prepares both layouts of x (natural [t,c] tiles AND transposed [c,t]
tiles) pre-arranged so every DMA is a plain contiguous burst — no on-device
transposes, no DMA-transpose mode, no 512B-descriptor stalls. The [ones,zeros]
pad columns (softmax-denominator trick) are baked into the natural tiles.

Per-core pipeline (window = 512 queries, kv = 1024 keys):
  scoresT[j,q] = kvT^T @ qT      (PE bf16, softmax scale folded into exp)
  expT = exp(scale*scoresT)      (ACT, PSUM->SBUF, bf16 out)
  causal mask: affine_select fill=0 on diagonal j-chunks; halo flag multiply
  o_unnorm[q, c+2] = sum_j expT[j,q]^T @ [kv | 1 | 0]  (PE; col 256 = denom)
  o = o_unnorm[:, :256] * recip(denom)   (DVE per-partition scalar)
  oT via PE transpose into PSUM; gating reads PSUM directly:
  u = sigmoid(a) * tanh(a)               (2x ACT + 1 DVE mul)
  res/skip[t,d] = u^T @ [Wr|Ws]^T        (PE, fused along N; GpSimd PSUM->SBUF
      bf16 copy), per-window batched DMA out in a pre-tiled layout the host
      unshuffles. Biases are added on the host (they are zero in this model).

The emission is software-pipelined with a one-window lag so the in-order
engine queues run window w+1's attention while ACT/DVE finish window w's
gating/projections. A few identity matmuls at the head of the Tensor queue
warm the HAM clock-gate during the initial DMA shadow.
"""

import numpy as np

B, T, C = 4, 4096, 256
W = 512                # attention window
TCH = T // 2           # rows per core
TH = TCH + W           # with halo
NWIN = TCH // W        # windows per core (4)
NBLK = TH // W         # 512-row blocks per core (5)
NCORES = 8

_CACHE = {}


def _build_program():
    import concourse.bacc as bacc
    import concourse.bass as bass
    import concourse.mybir as mybir
    import concourse.tile as tile
    from concourse.masks import make_identity

    f32 = mybir.dt.float32
    qdt = mybir.dt.bfloat16
    ts = bass.ts

    nc = bacc.Bacc("TRN2", target_bir_lowering=False, debug=False)

    xt_d = nc.dram_tensor("xt", [NBLK, 128, 2, W], qdt, kind="ExternalInput").ap()
    xn_d = nc.dram_tensor("xn", [NBLK, 128, 4, C + 2], qdt, kind="ExternalInput").ap()
    wc_d = nc.dram_tensor("wc", [128, 2, 2 * C], qdt, kind="ExternalInput").ap()
    hflag = nc.dram_tensor("hflag", [128, 1], f32, kind="ExternalInput").ap()
    rs_d = nc.dram_tensor("rs", [NWIN, 128, 4, 2 * C], qdt, kind="ExternalOutput").ap()

    Exp = mybir.ActivationFunctionType.Exp
    Tanh = mybir.ActivationFunctionType.Tanh
    Sig = mybir.ActivationFunctionType.Sigmoid

    with tile.TileContext(nc) as tc:
        with (
            tc.tile_pool(name="singles", bufs=1) as singles,
            tc.tile_pool(name="xn", bufs=5) as xn_pool,
            tc.tile_pool(name="xt", bufs=5) as xt_pool,
            tc.tile_pool(name="ex", bufs=16) as ex_pool,
            tc.tile_pool(name="on", bufs=6) as on_pool,
            tc.tile_pool(name="g", bufs=6) as g_pool,
            tc.tile_pool(name="outs", bufs=3) as out_pool,
            tc.tile_pool(name="small", bufs=8) as small,
            tc.tile_pool(name="psc", bufs=3, space="PSUM") as sc_pool,
            tc.tile_pool(name="pav", bufs=3, space="PSUM") as avj_pool,
            tc.tile_pool(name="pt", bufs=2, space="PSUM") as pt_pool,
        ):
            hf_sb = singles.tile([128, 1], f32)
            wc_sb = singles.tile([128, 2, 2 * C], qdt)
            xnb = [
                xn_pool.tile([128, 4, C + 2], qdt, tag="xn", name=f"xn{i}")
                for i in range(NBLK)
            ]
            xtb = [
                xt_pool.tile([128, 2, W], qdt, tag="xt", name=f"xt{i}")
                for i in range(NBLK)
            ]

            # weights/flag ride the scalar queue; x streams on sync, ordered
            # by first use. QK(0) needs blk1 (q) + blk0,1 (kv): cc0 halves
            # first so the cc=0 accumulation half starts earliest.
            nc.scalar.dma_start(out=wc_sb, in_=wc_d)
            nc.scalar.dma_start(out=hf_sb, in_=hflag)
            nc.sync.dma_start(out=xtb[1][:, 0, :], in_=xt_d[1, :, 0, :])
            nc.sync.dma_start(out=xtb[0][:, 0, :], in_=xt_d[0, :, 0, :])
            nc.sync.dma_start(out=xtb[1][:, 1, :], in_=xt_d[1, :, 1, :])
            nc.sync.dma_start(out=xtb[0][:, 1, :], in_=xt_d[0, :, 1, :])
            nc.sync.dma_start(out=xnb[0], in_=xn_d[0])
            nc.sync.dma_start(out=xnb[1], in_=xn_d[1])
            for blk in (2, 3, 4):
                nc.sync.dma_start(out=xtb[blk], in_=xt_d[blk])
                nc.sync.dma_start(out=xnb[blk], in_=xn_d[blk])

            identf = singles.tile([128, 128], f32)
            make_identity(nc, identf)
            ident = singles.tile([128, 128], qdt)
            nc.vector.tensor_copy(ident, identf)
            # hoist the ACT table load into the DMA shadow (Exp and Tanh
            # share a table set; Sigmoid does NOT — using it thrashes
            # ACT_TABLE_LOADs, so sigmoid comes from the tanh identity)
            actwarm = singles.tile([128, 2], f32)
            nc.scalar.activation(out=actwarm[:, 0:1], in_=identf[:, 0:1], func=Exp)
            nc.scalar.activation(out=actwarm[:, 1:2], in_=identf[:, 0:1], func=Tanh)
            # HAM clock-gate warm-up: real-rate matmuls on the identity while
            # the first x tiles are still in flight
            pwarm = pt_pool.tile([128, 128], f32, tag="pt")
            for k in range(4):
                nc.tensor.matmul(pwarm, ident, ident, start=(k == 0), stop=(k == 3))

            def attn_stage(w):
                """scores -> exp -> mask -> AV -> normalize -> oT (PSUM)."""
                qt = xtb[w + 1]

                # ---- scoresT[j, q] = (kv @ q^T) per j-chunk; exp; mask ----
                # chunks 0..5 full q; 6,7 only q in [256,512), one shared bank
                expts = [None] * 8  # (ap, q_lo) per j-chunk
                for jc in range(6):
                    q_lo = 128 if jc == 5 else 0  # q < 128 fully masked for 5
                    kvt = xtb[w + jc // 4]
                    psc = sc_pool.tile([128, W - q_lo], f32, tag="sc")
                    for cc in range(2):
                        nc.tensor.matmul(
                            psc,
                            kvt[:, cc, ts(jc % 4, 128)],
                            qt[:, cc, q_lo:W],
                            start=(cc == 0),
                            stop=(cc == 1),
                        )
                    ex = ex_pool.tile([128, W - q_lo], qdt, tag="ex2")
                    nc.scalar.activation(out=ex, in_=psc, func=Exp, scale=0.0625)
                    expts[jc] = (ex, q_lo)
                # 6,7 share one PSUM bank as a single accumulation group
                kvt = xtb[w + 1]
                psc = sc_pool.tile([128, 2, 256], f32, tag="sc")
                for i, jc in enumerate((6, 7)):
                    for cc in range(2):
                        nc.tensor.matmul(
                            psc[:, i, :],
                            kvt[:, cc, ts(jc % 4, 128)],
                            qt[:, cc, 256:512],
                            start=(i == 0 and cc == 0),
                            stop=(i == 1 and cc == 1),
                        )
                ex67 = ex_pool.tile([128, 2, 256], qdt, tag="ex1")
                nc.scalar.activation(out=ex67, in_=psc, func=Exp, scale=0.0625)
                expts[6] = (ex67[:, 0, :], 256)
                expts[7] = (ex67[:, 1, :], 256)

                # causal mask: valid iff q - p + 512 - jc*128 >= 0
                for jc in (4, 5):
                    ap, q_lo = expts[jc]
                    nc.gpsimd.affine_select(
                        out=ap,
                        in_=ap,
                        compare_op=mybir.AluOpType.is_ge,
                        fill=0.0,
                        base=q_lo + W - jc * 128,
                        channel_multiplier=-1,
                        pattern=[[1, W - q_lo]],
                    )
                nc.gpsimd.affine_select(
                    out=ex67,
                    in_=ex67,
                    compare_op=mybir.AluOpType.is_ge,
                    fill=0.0,
                    base=0,
                    channel_multiplier=-1,
                    pattern=[[-128, 2], [1, 256]],
                )
                if w == 0:
                    # halo validity flag (1.0 = real halo, 0.0 = first window)
                    for jc in range(4):
                        nc.vector.tensor_scalar_mul(
                            expts[jc][0], expts[jc][0], hf_sb
                        )

                # ---- AV + denom; normalize; transpose to oT (PSUM) ----
                pt4 = pt_pool.tile([128, 2, W], qdt, tag="pt")
                ons = []
                for qb in range(4):
                    jcs = [
                        jc
                        for jc in range(8)
                        if not (qb * 128 + 127) < (jc * 128 - W)
                    ]
                    pav = avj_pool.tile([128, C + 2], f32, tag="av")
                    for k, jc in enumerate(jcs):
                        ap, q_lo = expts[jc]
                        xn = xnb[w + jc // 4]
                        nc.tensor.matmul(
                            pav,
                            ap[:, qb * 128 - q_lo : qb * 128 - q_lo + 128],
                            xn[:, jc % 4, :],
                            start=(k == 0),
                            stop=(k == len(jcs) - 1),
                        )
                    rc = small.tile([128, 1], f32, tag="rc")
                    nc.vector.reciprocal(rc, pav[:, C : C + 1])
                    on = on_pool.tile([128, C], qdt, tag="on")
                    nc.vector.tensor_scalar_mul(on, pav[:, 0:C], rc)
                    ons.append(on)
                for qb in range(4):
                    for cc in range(2):
                        nc.tensor.transpose(
                            pt4[:, cc, ts(qb, 128)], ons[qb][:, ts(cc, 128)], ident
                        )
                return pt4

            def out_stage(w, pt4, last=False):
                """gating -> projections -> store, for window w.

                u = tanh(a) + tanh(a)*tanh(a/2); the 0.5 from
                sigmoid(a) = (1+tanh(a/2))/2 lives in the host-side weights.

                For the last window everything is cut per q-block so the
                gating -> proj -> copy -> store chain pipelines across
                engines instead of serializing into the drain tail.
                """
                rs_win = out_pool.tile([128, 4, 2 * C], qdt, tag="rs")
                qbs = [(qb, qb * 128) for qb in range(4)] if last else [(0, 0)]
                gw = W if not last else 128
                us_by_qb = {}
                for g_i, (qb, q0) in enumerate(qbs):
                    th2 = g_pool.tile([128, 2, gw], qdt, tag="th2")
                    ta = g_pool.tile([128, 2, gw], qdt, tag="ta")
                    src = pt4 if not last else pt4[:, :, q0 : q0 + 128]
                    nc.scalar.activation(out=th2, in_=src, func=Tanh, scale=0.5)
                    nc.scalar.activation(out=ta, in_=src, func=Tanh)
                    nc.vector.tensor_mul(th2, ta, th2)
                    nc.vector.tensor_add(th2, ta, th2)
                    us_by_qb[qb] = th2

                def proj(qb, u_t, u_off):
                    psp = avj_pool.tile([128, 2 * C], f32, tag="av")
                    for cc in range(2):
                        nc.tensor.matmul(
                            psp,
                            u_t[:, cc, u_off : u_off + 128],
                            wc_sb[:, cc, :],
                            start=(cc == 0),
                            stop=(cc == 1),
                        )
                    # PSUM->SBUF move (GpSimd/DMA can't touch PSUM): mostly
                    # DVE, one per window on ACT to balance engine load
                    if qb == 1:
                        nc.scalar.activation(
                            out=rs_win[:, qb, :],
                            in_=psp,
                            func=mybir.ActivationFunctionType.Copy,
                        )
                    else:
                        nc.vector.tensor_copy(rs_win[:, qb, :], psp)

                if last:
                    for qb in range(4):
                        proj(qb, us_by_qb[qb], 0)
                        q = nc.sync if qb % 2 == 0 else nc.scalar
                        q.dma_start(out=rs_d[w, :, qb, :], in_=rs_win[:, qb, :])
                else:
                    u_t = us_by_qb[0]
                    for qb in range(4):
                        proj(qb, u_t, qb * 128)
                    nc.sync.dma_start(out=rs_d[w], in_=rs_win)

            # software pipeline with a one-window lag
            pts = {}
            pts[0] = attn_stage(0)
            for w in range(1, NWIN):
                pts[w] = attn_stage(w)
                out_stage(w - 1, pts.pop(w - 1))
            out_stage(NWIN - 1, pts.pop(NWIN - 1), last=True)

    nc.compile()
    return nc


def _get_program():
    if "nc" not in _CACHE:
        _CACHE["nc"] = _build_program()
    return _CACHE["nc"]


def _make_in_maps(x, Wr, br, Ws, bs):
    import ml_dtypes

    bf16 = ml_dtypes.bfloat16
    x = np.asarray(x, dtype=np.float32)
    Wr = np.asarray(Wr, dtype=np.float32)
    Ws = np.asarray(Ws, dtype=np.float32)

    # 0.5x from the sigmoid(a) = (1 + tanh(a/2))/2 identity folded into
    # weights; res/skip projections fused along the output dim
    wcomb = 0.5 * np.concatenate([Wr.T, Ws.T], axis=1)    # [C, 2C]
    wc_h = np.ascontiguousarray(
        wcomb.reshape(2, 128, 2 * C).transpose(1, 0, 2)
    ).astype(bf16)

    in_maps = []
    for i in range(NCORES):
        b, h = divmod(i, 2)
        xhf = np.empty((TH, C), np.float32)
        if h == 0:
            xhf[:W] = 0.0
            flag = np.zeros((128, 1), np.float32)
        else:
            xhf[:W] = x[b, TCH - W : TCH]
            flag = np.ones((128, 1), np.float32)
        xhf[W:] = x[b, h * TCH : (h + 1) * TCH]
        xb = xhf.astype(bf16)
        # transposed tiles: xt[blk, p, cc, t] = x[blk*512 + t, cc*128 + p]
        xt_h = np.ascontiguousarray(
            xb.reshape(NBLK, W, 2, 128).transpose(0, 3, 2, 1)
        )
        # natural tiles + baked [ones, zeros] pad columns (denominator trick)
        xn_core = xb.reshape(NBLK, 4, 128, C).transpose(0, 2, 1, 3)
        xn_h = np.empty((NBLK, 128, 4, C + 2), bf16)
        xn_h[:, :, :, 0:C] = xn_core
        xn_h[:, :, :, C] = bf16(1.0)
        xn_h[:, :, :, C + 1] = bf16(0.0)
        in_maps.append(
            {
                "xt": xt_h,
                "xn": np.ascontiguousarray(xn_h),
                "wc": wc_h,
                "hflag": flag,
            }
        )
    return in_maps


def _gather(results, br, bs):
    residual = np.empty((B, T, C), np.float32)
    skip = np.empty((B, T, C), np.float32)
    for i in range(NCORES):
        b, h = divmod(i, 2)
        rs = np.asarray(results[i]["rs"], dtype=np.float32)  # [4, 128, 4, 2C]
        rs = rs.transpose(0, 2, 1, 3).reshape(TCH, 2 * C)
        residual[b, h * TCH : (h + 1) * TCH] = rs[:, 0:C]
        skip[b, h * TCH : (h + 1) * TCH] = rs[:, C : 2 * C]
    br = np.asarray(br, dtype=np.float32)
    bs = np.asarray(bs, dtype=np.float32)
    if br.any():
        residual += br
    if bs.any():
        skip += bs
    return residual, skip


def kernel(x, Wr, br, Ws, bs):
    from concourse.bass_utils import run_bass_kernel_spmd

    nc = _get_program()
    in_maps = _make_in_maps(x, Wr, br, Ws, bs)
    res = run_bass_kernel_spmd(nc, in_maps, list(range(NCORES)))
    return _gather(res.results, br, bs)


# revision 14
# speedup vs baseline: 1.2394x; 1.1200x over previous
"""Trainium2 Bass kernel for nn_AttentionResBlock (windowed causal attention +
sigmoid*tanh gating + two 1x1 convs), SPMD over 8 NeuronCores.

Sharding: data-parallel over (batch, sequence-half): core i handles batch i//2,
rows [h*2048, (h+1)*2048) with h = i%2, plus a 512-row halo (previous window;
zeros + mask flag for h==0). No cross-core communication.

Host prepares both layouts of x (natural [t,c] tiles AND transposed [c,t]
tiles) pre-arranged so every DMA is a plain contiguous burst — no on-device
transposes, no DMA-transpose mode, no 512B-descriptor stalls. The [ones,zeros]
pad columns (softmax-denominator trick) are baked into the natural tiles.

Per-core pipeline (window = 512 queries, kv = 1024 keys):
  scoresT[j,q] = kvT^T @ qT      (PE bf16, softmax scale folded into exp)
  expT = exp(scale*scoresT)      (ACT, PSUM->SBUF, bf16 out)
  causal mask: affine_select fill=0 on diagonal j-chunks; halo flag multiply
  o_unnorm[q, c+2] = sum_j expT[j,q]^T @ [kv | 1 | 0]  (PE; col 256 = denom)
  o = o_unnorm[:, :256] * recip(denom)   (DVE per-partition scalar)
  oT via PE transpose into PSUM; gating reads PSUM directly:
  u = sigmoid(a) * tanh(a)               (2x ACT + 1 DVE mul)
  res/skip[t,d] = u^T @ [Wr|Ws]^T        (PE, fused along N; GpSimd PSUM->SBUF
      bf16 copy), per-window batched DMA out in a pre-tiled layout the host
      unshuffles. Biases are added on the host (they are zero in this model).

The emission is software-pipelined with a one-window lag so the in-order
engine queues run window w+1's attention while ACT/DVE finish window w's
gating/projections. A few identity matmuls at the head of the Tensor queue
warm the HAM clock-gate during the initial DMA shadow.
"""

import numpy as np

B, T, C = 4, 4096, 256
W = 512                # attention window
TCH = T // 2           # rows per core
TH = TCH + W           # with halo
NWIN = TCH // W        # windows per core (4)
NBLK = TH // W         # 512-row blocks per core (5)
NCORES = 8

_CACHE = {}


def _build_program():
    import concourse.bacc as bacc
    import concourse.bass as bass
    import concourse.mybir as mybir
    import concourse.tile as tile
    from concourse.masks import make_identity

    f32 = mybir.dt.float32
    qdt = mybir.dt.bfloat16
    f8 = mybir.dt.float8e4
    DR = mybir.MatmulPerfMode.DoubleRow
    ts = bass.ts

    nc = bacc.Bacc("TRN2", target_bir_lowering=False, debug=False)

    xt_d = nc.dram_tensor("xt", [NBLK, 128, 2, W], f8, kind="ExternalInput").ap()
    xn_d = nc.dram_tensor("xn", [NBLK, 128, 4, C + 2], qdt, kind="ExternalInput").ap()
    wc_d = nc.dram_tensor("wc", [128, 2, 2 * C], qdt, kind="ExternalInput").ap()
    hflag = nc.dram_tensor("hflag", [128, 1], f32, kind="ExternalInput").ap()
    rs_d = nc.dram_tensor("rs", [NWIN, 128, 4, 2 * C], qdt, kind="ExternalOutput").ap()

    Exp = mybir.ActivationFunctionType.Exp
    Tanh = mybir.ActivationFunctionType.Tanh
    Sig = mybir.ActivationFunctionType.Sigmoid

    with tile.TileContext(nc) as tc:
        with (
            tc.tile_pool(name="singles", bufs=1) as singles,
            tc.tile_pool(name="xn", bufs=5) as xn_pool,
            tc.tile_pool(name="xt", bufs=5) as xt_pool,
            tc.tile_pool(name="ex", bufs=16) as ex_pool,
            tc.tile_pool(name="on", bufs=6) as on_pool,
            tc.tile_pool(name="g", bufs=6) as g_pool,
            tc.tile_pool(name="outs", bufs=3) as out_pool,
            tc.tile_pool(name="small", bufs=8) as small,
            tc.tile_pool(name="psc", bufs=3, space="PSUM") as sc_pool,
            tc.tile_pool(name="pav", bufs=3, space="PSUM") as avj_pool,
            tc.tile_pool(name="pt", bufs=2, space="PSUM") as pt_pool,
        ):
            hf_sb = singles.tile([128, 1], f32)
            wc_sb = singles.tile([128, 2, 2 * C], qdt)
            xnb = [
                xn_pool.tile([128, 4, C + 2], qdt, tag="xn", name=f"xn{i}")
                for i in range(NBLK)
            ]
            xtb = [
                xt_pool.tile([128, 2, W], f8, tag="xt", name=f"xt{i}")
                for i in range(NBLK)
            ]

            # weights/flag ride the scalar queue; x streams on sync, ordered
            # by first use. QK(0) needs blk1 (q) + blk0,1 (kv): cc0 halves
            # first so the cc=0 accumulation half starts earliest.
            nc.scalar.dma_start(out=wc_sb, in_=wc_d)
            nc.scalar.dma_start(out=hf_sb, in_=hflag)
            nc.sync.dma_start(out=xtb[1][:, 0, :], in_=xt_d[1, :, 0, :])
            nc.sync.dma_start(out=xtb[0][:, 0, :], in_=xt_d[0, :, 0, :])
            nc.sync.dma_start(out=xtb[1][:, 1, :], in_=xt_d[1, :, 1, :])
            nc.sync.dma_start(out=xtb[0][:, 1, :], in_=xt_d[0, :, 1, :])
            nc.sync.dma_start(out=xnb[0], in_=xn_d[0])
            nc.sync.dma_start(out=xnb[1], in_=xn_d[1])
            for blk in (2, 3, 4):
                nc.sync.dma_start(out=xtb[blk], in_=xt_d[blk])
                nc.sync.dma_start(out=xnb[blk], in_=xn_d[blk])

            identf = singles.tile([128, 128], f32)
            make_identity(nc, identf)
            ident = singles.tile([128, 128], qdt)
            nc.vector.tensor_copy(ident, identf)
            # hoist the ACT table load into the DMA shadow (Exp and Tanh
            # share a table set; Sigmoid does NOT — using it thrashes
            # ACT_TABLE_LOADs, so sigmoid comes from the tanh identity)
            actwarm = singles.tile([128, 2], f32)
            nc.scalar.activation(out=actwarm[:, 0:1], in_=identf[:, 0:1], func=Exp)
            nc.scalar.activation(out=actwarm[:, 1:2], in_=identf[:, 0:1], func=Tanh)
            # HAM clock-gate warm-up: real-rate matmuls on the identity while
            # the first x tiles are still in flight
            pwarm = pt_pool.tile([128, 128], f32, tag="pt")
            for k in range(4):
                nc.tensor.matmul(pwarm, ident, ident, start=(k == 0), stop=(k == 3))

            def attn_stage(w):
                """scores -> exp -> mask -> AV -> normalize -> oT (PSUM)."""
                qt = xtb[w + 1]

                # ---- scoresT[j, q] = (kv @ q^T) per j-chunk; exp; mask ----
                # fp8 DoubleRow: the [128, 2, *] cc-chunked tiles are exactly
                # the two-k-tile layout, so each j-chunk is ONE double-pumped
                # matmul over the full 256-channel contraction.
                # chunks 0..5 full q; 6,7 only q in [256,512), one shared bank
                expts = [None] * 8  # (ap, q_lo) per j-chunk
                for jc in range(6):
                    q_lo = 128 if jc == 5 else 0  # q < 128 fully masked for 5
                    kvt = xtb[w + jc // 4]
                    psc = sc_pool.tile([128, W - q_lo], f32, tag="sc")
                    nc.tensor.matmul(
                        psc,
                        kvt[:, :, ts(jc % 4, 128)],
                        qt[:, :, q_lo:W],
                        start=True,
                        stop=True,
                        perf_mode=DR,
                    )
                    ex = ex_pool.tile([128, W - q_lo], qdt, tag="ex2")
                    nc.scalar.activation(out=ex, in_=psc, func=Exp, scale=0.0625)
                    expts[jc] = (ex, q_lo)
                # 6,7 share one PSUM bank as a single accumulation group
                kvt = xtb[w + 1]
                psc = sc_pool.tile([128, 2, 256], f32, tag="sc")
                for i, jc in enumerate((6, 7)):
                    nc.tensor.matmul(
                        psc[:, i, :],
                        kvt[:, :, ts(jc % 4, 128)],
                        qt[:, :, 256:512],
                        start=(i == 0),
                        stop=(i == 1),
                        perf_mode=DR,
                    )
                ex67 = ex_pool.tile([128, 2, 256], qdt, tag="ex1")
                nc.scalar.activation(out=ex67, in_=psc, func=Exp, scale=0.0625)
                expts[6] = (ex67[:, 0, :], 256)
                expts[7] = (ex67[:, 1, :], 256)

                # causal mask: valid iff q - p + 512 - jc*128 >= 0
                for jc in (4, 5):
                    ap, q_lo = expts[jc]
                    nc.gpsimd.affine_select(
                        out=ap,
                        in_=ap,
                        compare_op=mybir.AluOpType.is_ge,
                        fill=0.0,
                        base=q_lo + W - jc * 128,
                        channel_multiplier=-1,
                        pattern=[[1, W - q_lo]],
                    )
                nc.gpsimd.affine_select(
                    out=ex67,
                    in_=ex67,
                    compare_op=mybir.AluOpType.is_ge,
                    fill=0.0,
                    base=0,
                    channel_multiplier=-1,
                    pattern=[[-128, 2], [1, 256]],
                )
                if w == 0:
                    # halo validity flag (1.0 = real halo, 0.0 = first window)
                    for jc in range(4):
                        nc.vector.tensor_scalar_mul(
                            expts[jc][0], expts[jc][0], hf_sb
                        )

                # ---- AV + denom; normalize; transpose to oT (PSUM) ----
                pt4 = pt_pool.tile([128, 2, W], qdt, tag="pt")
                ons = []
                for qb in range(4):
                    jcs = [
                        jc
                        for jc in range(8)
                        if not (qb * 128 + 127) < (jc * 128 - W)
                    ]
                    pav = avj_pool.tile([128, C + 2], f32, tag="av")
                    for k, jc in enumerate(jcs):
                        ap, q_lo = expts[jc]
                        xn = xnb[w + jc // 4]
                        nc.tensor.matmul(
                            pav,
                            ap[:, qb * 128 - q_lo : qb * 128 - q_lo + 128],
                            xn[:, jc % 4, :],
                            start=(k == 0),
                            stop=(k == len(jcs) - 1),
                        )
                    rc = small.tile([128, 1], f32, tag="rc")
                    nc.vector.reciprocal(rc, pav[:, C : C + 1])
                    on = on_pool.tile([128, C], qdt, tag="on")
                    nc.vector.tensor_scalar_mul(on, pav[:, 0:C], rc)
                    ons.append(on)
                for qb in range(4):
                    for cc in range(2):
                        nc.tensor.transpose(
                            pt4[:, cc, ts(qb, 128)], ons[qb][:, ts(cc, 128)], ident
                        )
                return pt4

            def out_stage(w, pt4, last=False):
                """gating -> projections -> store, for window w.

                u = tanh(a) + tanh(a)*tanh(a/2); the 0.5 from
                sigmoid(a) = (1+tanh(a/2))/2 lives in the host-side weights.

                For the last window everything is cut per q-block so the
                gating -> proj -> copy -> store chain pipelines across
                engines instead of serializing into the drain tail.
                """
                rs_win = out_pool.tile([128, 4, 2 * C], qdt, tag="rs")
                qbs = [(qb, qb * 128) for qb in range(4)] if last else [(0, 0)]
                gw = W if not last else 128
                us_by_qb = {}
                for g_i, (qb, q0) in enumerate(qbs):
                    th2 = g_pool.tile([128, 2, gw], qdt, tag="th2")
                    ta = g_pool.tile([128, 2, gw], qdt, tag="ta")
                    src = pt4 if not last else pt4[:, :, q0 : q0 + 128]
                    nc.scalar.activation(out=th2, in_=src, func=Tanh, scale=0.5)
                    nc.scalar.activation(out=ta, in_=src, func=Tanh)
                    nc.vector.tensor_mul(th2, ta, th2)
                    nc.vector.tensor_add(th2, ta, th2)
                    us_by_qb[qb] = th2

                def proj(qb, u_t, u_off):
                    psp = avj_pool.tile([128, 2 * C], f32, tag="av")
                    for cc in range(2):
                        nc.tensor.matmul(
                            psp,
                            u_t[:, cc, u_off : u_off + 128],
                            wc_sb[:, cc, :],
                            start=(cc == 0),
                            stop=(cc == 1),
                        )
                    # PSUM->SBUF move (GpSimd/DMA can't touch PSUM): mostly
                    # DVE, one per window on ACT to balance engine load
                    if qb == 1:
                        nc.scalar.activation(
                            out=rs_win[:, qb, :],
                            in_=psp,
                            func=mybir.ActivationFunctionType.Copy,
                        )
                    else:
                        nc.vector.tensor_copy(rs_win[:, qb, :], psp)

                if last:
                    for qb in range(4):
                        proj(qb, us_by_qb[qb], 0)
                        q = nc.sync if qb % 2 == 0 else nc.scalar
                        q.dma_start(out=rs_d[w, :, qb, :], in_=rs_win[:, qb, :])
                else:
                    u_t = us_by_qb[0]
                    for qb in range(4):
                        proj(qb, u_t, qb * 128)
                    nc.sync.dma_start(out=rs_d[w], in_=rs_win)

            # software pipeline with a one-window lag
            pts = {}
            pts[0] = attn_stage(0)
            for w in range(1, NWIN):
                pts[w] = attn_stage(w)
                out_stage(w - 1, pts.pop(w - 1))
            out_stage(NWIN - 1, pts.pop(NWIN - 1), last=True)

    nc.compile()
    return nc


def _get_program():
    if "nc" not in _CACHE:
        _CACHE["nc"] = _build_program()
    return _CACHE["nc"]


def _make_in_maps(x, Wr, br, Ws, bs):
    import ml_dtypes

    bf16 = ml_dtypes.bfloat16
    x = np.asarray(x, dtype=np.float32)
    Wr = np.asarray(Wr, dtype=np.float32)
    Ws = np.asarray(Ws, dtype=np.float32)

    # 0.5x from the sigmoid(a) = (1 + tanh(a/2))/2 identity folded into
    # weights; res/skip projections fused along the output dim
    wcomb = 0.5 * np.concatenate([Wr.T, Ws.T], axis=1)    # [C, 2C]
    wc_h = np.ascontiguousarray(
        wcomb.reshape(2, 128, 2 * C).transpose(1, 0, 2)
    ).astype(bf16)

    in_maps = []
    for i in range(NCORES):
        b, h = divmod(i, 2)
        xhf = np.empty((TH, C), np.float32)
        if h == 0:
            xhf[:W] = 0.0
            flag = np.zeros((128, 1), np.float32)
        else:
            xhf[:W] = x[b, TCH - W : TCH]
            flag = np.ones((128, 1), np.float32)
        xhf[W:] = x[b, h * TCH : (h + 1) * TCH]
        xb = xhf.astype(bf16)
        # transposed tiles: xt[blk, p, cc, t] = x[blk*512 + t, cc*128 + p]
        # (fp8 e4m3: QK runs double-pumped; |x| ~ 5 max, far below 240)
        xt_h = np.ascontiguousarray(
            xhf.reshape(NBLK, W, 2, 128).transpose(0, 3, 2, 1)
        ).astype(ml_dtypes.float8_e4m3)
        # natural tiles + baked [ones, zeros] pad columns (denominator trick)
        xn_core = xb.reshape(NBLK, 4, 128, C).transpose(0, 2, 1, 3)
        xn_h = np.empty((NBLK, 128, 4, C + 2), bf16)
        xn_h[:, :, :, 0:C] = xn_core
        xn_h[:, :, :, C] = bf16(1.0)
        xn_h[:, :, :, C + 1] = bf16(0.0)
        in_maps.append(
            {
                "xt": xt_h,
                "xn": np.ascontiguousarray(xn_h),
                "wc": wc_h,
                "hflag": flag,
            }
        )
    return in_maps


def _gather(results, br, bs):
    residual = np.empty((B, T, C), np.float32)
    skip = np.empty((B, T, C), np.float32)
    for i in range(NCORES):
        b, h = divmod(i, 2)
        rs = np.asarray(results[i]["rs"], dtype=np.float32)  # [4, 128, 4, 2C]
        rs = rs.transpose(0, 2, 1, 3).reshape(TCH, 2 * C)
        residual[b, h * TCH : (h + 1) * TCH] = rs[:, 0:C]
        skip[b, h * TCH : (h + 1) * TCH] = rs[:, C : 2 * C]
    br = np.asarray(br, dtype=np.float32)
    bs = np.asarray(bs, dtype=np.float32)
    if br.any():
        residual += br
    if bs.any():
        skip += bs
    return residual, skip


def kernel(x, Wr, br, Ws, bs):
    from concourse.bass_utils import run_bass_kernel_spmd

    nc = _get_program()
    in_maps = _make_in_maps(x, Wr, br, Ws, bs)
    res = run_bass_kernel_spmd(nc, in_maps, list(range(NCORES)))
    return _gather(res.results, br, bs)
